# revision 31
# baseline (speedup 1.0000x reference)
"""DeepSeek-V3 MoE gate (sigmoid + group-restricted top-k routing) on 8 TRN2
NeuronCores.

Strategy (data-parallel over tokens, per sharding hint):
  - x [16384, 7168] f32 is sharded 2048 tokens/core; weight [256, 7168] and
    bias [256] are replicated.
  - logits = x @ w.T at ~fp32 precision via an fp16 hi/lo 3-term split
    (x*2^8 = xh+xl, w*2^12 = wh+wl; logits*2^20 = xh@wh + xh@wl + xl@wh).
    The w split AND its transposed blocked layout are precomputed on the
    host (small, replicated weight prep — saves all on-device W transposes
    and the startup serialization they caused).  Per 128-token tile the hi
    part of x is transposed on the PE (fp16 rate, PSUM-staged, ACT evicts)
    and the lo part by the DMA xbar, so the PE spends ~86% of its time on
    the 168 matmuls.  The hi terms interleave per d-tile in one PSUM bank
    and the lo term accumulates in its own bank before an ACT-copy + DVE
    add, reproducing the reference-matching accumulation order bit-exactly
    (0 idx mismatches on the test input).
  - Sigmoid (ACT LUT with the 2^-20 descale) then group-limited top-8
    selection + weight normalization on the DVE via max8/max_index/
    match_replace, software-pipelined one tile behind the GEMM.
  - Queue discipline matters in TimelineSim: DMA dispatches hold the
    issuing engine's SEQ through their semaphore waits, so x loads + lo
    xbars live on the SP queue (waits satisfied at dispatch), W loads on
    the ACT HWDGE queue, outputs on the gpsimd SWDGE queue.
  - Outputs (w [16384,8] f32, idx [16384,8] i32) are gathered host-side.
"""
import numpy as np

import concourse.bass as bass
import concourse.mybir as mybir
import concourse.tile as tile
from concourse import bacc
from concourse.bass_utils import run_bass_kernel_spmd

F32 = mybir.dt.float32
F16 = mybir.dt.float16
U32 = mybir.dt.uint32
I32 = mybir.dt.int32
AF = mybir.ActivationFunctionType
ALU = mybir.AluOpType
AX = mybir.AxisListType

N_CORES = 8
T = 16384
D = 7168
E = 256
TOPK = 8
N_GROUPS = 8
GSIZE = E // N_GROUPS       # 32
ROUTE_SCALE = 2.5

TPC = T // N_CORES          # 2048 tokens per core
NT = TPC // 128             # 16 tiles of 128 tokens
HALF = D // 2               # 3584
NDH = HALF // 128           # 28 d-tiles per half
ND = D // 128               # 56
GRP = 7                     # d-tiles per PE-transpose group (one PSUM bank)

XSCALE = 2.0 ** 8
WSCALE = 2.0 ** 12
DESCALE = 1.0 / (XSCALE * WSCALE)

NEG_MASK = -1.0e30
MARK = -3.0

_CACHE = {}


def _routing_thunks(nc, rt, scores, biasr, OW, OI, i):
    """Group-limited top-8 routing for one tile of 128 tokens, returned as a
    list of emission thunks so the caller can place them in the next tile's
    emission (DVE work overlaps the next tile's GEMM)."""
    st = {}

    def t_s():
        st["s"] = rt.tile([128, E], F32, tag="s", name="s", bufs=1)
        nc.vector.tensor_add(st["s"][:], scores[:], biasr[:])

    def t_gmax1():
        st["gmax1"] = rt.tile([128, 8], F32, tag="gmax1", name="gmax1")
        nc.vector.tensor_reduce(
            st["gmax1"][:], st["s"].rearrange("p (g k) -> p g k", k=GSIZE),
            axis=AX.X, op=ALU.max,
        )

    def t_scr():
        st["scr"] = rt.tile([128, E], F32, tag="scr", name="scr", bufs=1)
        nc.vector.match_replace(
            st["scr"][:], in_to_replace=st["gmax1"][:], in_values=st["s"][:],
            imm_value=MARK,
        )

    def t_gsum():
        gmax2 = rt.tile([128, 8], F32, tag="gmax2", name="gmax2")
        nc.vector.tensor_reduce(
            gmax2[:], st["scr"].rearrange("p (g k) -> p g k", k=GSIZE),
            axis=AX.X, op=ALU.max,
        )
        st["gsum"] = rt.tile([128, 8], F32, tag="gsum", name="gsum")
        nc.vector.tensor_add(st["gsum"][:], st["gmax1"][:], gmax2[:])

    def t_pen():
        g8 = rt.tile([128, 8], F32, tag="g8", name="g8")
        nc.vector.max(g8[:], st["gsum"][:])
        st["pen"] = rt.tile([128, 8], F32, tag="pen", name="pen")
        nc.vector.tensor_scalar(
            st["pen"][:], st["gsum"][:], g8[:, 3:4], scalar2=NEG_MASK,
            op0=ALU.is_lt, op1=ALU.mult,
        )

    def t_masked():
        st["masked"] = rt.tile([128, E], F32, tag="masked", name="masked", bufs=1)
        pen3 = st["pen"].rearrange("p (g k) -> p g k", k=1).to_broadcast(
            [128, N_GROUPS, GSIZE]
        )
        nc.vector.tensor_tensor(
            st["masked"].rearrange("p (g k) -> p g k", k=GSIZE),
            st["s"].rearrange("p (g k) -> p g k", k=GSIZE), pen3, op=ALU.add,
        )

    def t_sel8():
        st["sel8"] = rt.tile([128, 8], F32, tag="sel8", name="sel8")
        nc.vector.max(st["sel8"][:], st["masked"][:])

    def t_idx8():
        st["idx8"] = rt.tile([128, 8], U32, tag="idx8", name="idx8")
        nc.vector.max_index(st["idx8"][:], st["sel8"][:], st["masked"][:])

    def t_scr2():
        st["scr2"] = rt.tile([128, E], F32, tag="scr2", name="scr2", bufs=1)
        nc.vector.match_replace(
            st["scr2"][:], in_to_replace=st["sel8"][:], in_values=st["masked"][:],
            imm_value=MARK,
        )

    def t_mark():
        st["mark"] = rt.tile([128, E], F32, tag="mark", name="mark", bufs=1)
        nc.vector.tensor_scalar(
            st["mark"][:], st["scr2"][:], MARK, scalar2=None, op0=ALU.is_equal
        )

    def t_dsc():
        st["dsc"] = rt.tile([128, E], F32, tag="dsc", name="dsc", bufs=1)
        nc.vector.tensor_tensor(st["dsc"][:], scores[:], st["mark"][:], op=ALU.mult)

    def t_ssel8():
        st["ssel8"] = rt.tile([128, 8], F32, tag="ssel8", name="ssel8")
        nc.vector.max(st["ssel8"][:], st["dsc"][:])

    def t_isel8():
        st["isel8"] = rt.tile([128, 8], U32, tag="isel8", name="isel8")
        nc.vector.max_index(st["isel8"][:], st["ssel8"][:], st["dsc"][:])

    def t_casts():
        st["idx8f"] = rt.tile([128, 8], F32, tag="idx8f", name="idx8f")
        nc.vector.tensor_copy(st["idx8f"][:], st["idx8"][:])
        st["isel8f"] = rt.tile([128, 8], F32, tag="isel8f", name="isel8f")
        nc.vector.tensor_copy(st["isel8f"][:], st["isel8"][:])

    def t_eq():
        st["eq"] = rt.tile([128, 8, 8], F32, tag="eq", name="eq", bufs=1)
        idx8_b = st["idx8f"].rearrange("p (j k) -> p j k", k=1).to_broadcast(
            [128, 8, 8]
        )
        isel8_b = st["isel8f"].rearrange("p (k j) -> p k j", k=1).to_broadcast(
            [128, 8, 8]
        )
        nc.vector.tensor_tensor(st["eq"][:], idx8_b, isel8_b, op=ALU.is_equal)

    def t_wj():
        prod = rt.tile([128, 8, 8], F32, tag="prod", name="prod", bufs=1)
        ssel8_b = st["ssel8"].rearrange("p (k j) -> p k j", k=1).to_broadcast(
            [128, 8, 8]
        )
        nc.vector.tensor_tensor(prod[:], st["eq"][:], ssel8_b, op=ALU.mult)
        st["wj"] = rt.tile([128, 8], F32, tag="wj", name="wj")
        nc.vector.tensor_reduce(st["wj"][:], prod[:], axis=AX.X, op=ALU.add)

    def t_rec():
        sumw = rt.tile([128, 1], F32, tag="sumw", name="sumw")
        nc.vector.tensor_reduce(sumw[:], st["wj"][:], axis=AX.X, op=ALU.add)
        st["rec"] = rt.tile([128, 1], F32, tag="rec", name="rec")
        nc.vector.reciprocal(st["rec"][:], sumw[:])

    def t_out():
        wout = rt.tile([128, TOPK], F32, tag="wout", name="wout")
        nc.vector.tensor_scalar(
            wout[:], st["wj"][:], st["rec"][:, 0:1], scalar2=ROUTE_SCALE,
            op0=ALU.mult, op1=ALU.mult,
        )
        iout = rt.tile([128, TOPK], I32, tag="iout", name="iout")
        nc.vector.tensor_copy(iout[:], st["idx8"][:])
        nc.gpsimd.dma_start(OW[bass.ts(i, 128), :], wout[:])
        nc.gpsimd.dma_start(OI[bass.ts(i, 128), :], iout[:])

    return [t_s, t_gmax1, t_scr, t_gsum, t_pen, t_masked, t_sel8, t_idx8,
            t_scr2, t_mark, t_dsc, t_ssel8, t_isel8, t_casts, t_eq, t_wj,
            t_rec, t_out]


def _build(hi_xbar=frozenset(), lo_pe=frozenset(), evict_dve_halves=frozenset(),
           hi_pek=14, hi_pek_tiles=()):
    """3-term fp16 GEMM with host-prepped transposed W halves.

    hi_xbar: set of (tile, half) whose hi-part transpose goes entirely through
    the DMA xbar instead of the PE.  lo_pe: set of (tile, half) whose lo-part
    transpose goes on the PE instead of the xbar.  evict_dve_halves: set of
    (tile, half) whose PE-transpose PSUM evictions run on the DVE instead of
    ACT.  hi_pek: of each half's 28 hi d-tiles, how many are PE-transposed
    (rest go via xbar); hi_pek_tiles maps tile -> override (e.g. 28 for the
    DMA-starved first tiles).
    """
    hi_pek_tiles = dict(hi_pek_tiles)
    nc = bacc.Bacc("TRN2", target_bir_lowering=False, debug=False)

    X = nc.dram_tensor("X", [TPC, D], F32, kind="ExternalInput")
    # host-transposed fp16 weight halves, blocked [128, 56, 256]
    WHT = nc.dram_tensor("WHT", [128, ND * E], F16, kind="ExternalInput")
    WLT = nc.dram_tensor("WLT", [128, ND * E], F16, kind="ExternalInput")
    BIASR = nc.dram_tensor("BIASR", [128, E], F32, kind="ExternalInput")
    IDENT16 = nc.dram_tensor("IDENT16", [128, 128], F16, kind="ExternalInput")
    OW = nc.dram_tensor("OW", [TPC, TOPK], F32, kind="ExternalOutput")
    OI = nc.dram_tensor("OI", [TPC, TOPK], I32, kind="ExternalOutput")

    with tile.TileContext(nc) as tc:
        with (
            tc.tile_pool(name="consts", bufs=1) as consts,
            tc.tile_pool(name="wtp", bufs=1) as wtp,
            tc.tile_pool(name="xin", bufs=2) as xin,
            tc.tile_pool(name="xnh", bufs=2) as xnhp,
            tc.tile_pool(name="xnl", bufs=2) as xnlp,
            tc.tile_pool(name="xth", bufs=1) as xthp,
            tc.tile_pool(name="xtl", bufs=1) as xtlp,
            tc.tile_pool(name="rt", bufs=2) as rt,
            tc.tile_pool(name="pst", bufs=5, space="PSUM") as pst,
            tc.tile_pool(name="psl", bufs=2, space="PSUM") as psl,
        ):
            # ---- pre-loop: first x tile, W chunks, consts --------------
            identh = consts.tile([128, 128], F16)
            nc.sync.dma_start(identh[:], IDENT16[:])
            biasr = consts.tile([128, E], F32)
            nc.sync.dma_start(biasr[:], BIASR[:])

            x0 = []
            for h in range(2):
                xh_t = xin.tile([128, HALF], F32, tag=f"xn{h}", name="xn")
                nc.sync.dma_start(xh_t[:], X[0:128, bass.ts(h, HALF)])
                x0.append(xh_t)

            # W on the SWDGE queue, chunked so the first matmuls can start
            # before the whole weight landed; x tile 1 is slotted between the
            # hi and lo weight halves so tile 1's casts aren't starved
            wht = wtp.tile([128, ND * E], F16)
            wlt = wtp.tile([128, ND * E], F16)
            WCH = 2
            for q in range(WCH):
                nc.scalar.dma_start(
                    wht[:, bass.ts(q, ND * E // WCH)],
                    WHT[:, bass.ts(q, ND * E // WCH)],
                )
            for q in range(WCH):
                nc.scalar.dma_start(
                    wlt[:, bass.ts(q, ND * E // WCH)],
                    WLT[:, bass.ts(q, ND * E // WCH)],
                )
            x1 = []
            for h in range(2):
                xh_t = xin.tile([128, HALF], F32, tag=f"xn{h}", name="xn")
                nc.sync.dma_start(xh_t[:], X[128:256, bass.ts(h, HALF)])
                x1.append(xh_t)

            def emit_casts(i, xhs):
                """hi cast (ACT) + lo residual (DVE) for tile i; returns
                (xnh, xnl) per half."""
                nh, nl = [], []
                for h in range(2):
                    xnh = xnhp.tile([128, HALF], F16, tag=f"xnh{h}", name="xnh")
                    nc.scalar.activation(xnh[:], xhs[h][:], AF.Copy, scale=XSCALE)
                    nh.append(xnh)
                for h in range(2):
                    xnl = xnlp.tile([128, HALF], F16, tag=f"xnl{h}", name="xnl")
                    nc.vector.scalar_tensor_tensor(
                        xnl[:], xhs[h][:], XSCALE, nh[h][:],
                        op0=ALU.mult, op1=ALU.subtract,
                    )
                    nl.append(xnl)
                return nh, nl

            def emit_transpose(i, h, src, dst_tag, pek, evict_dve):
                """Transpose one [128, HALF] fp16 half into [128, NDH, 128].
                The first `pek` d-tiles go via the PE (+PSUM evict), the rest
                through the DMA xbar."""
                pool = xthp if dst_tag == "xhT" else xtlp
                xt = pool.tile([128, NDH, 128], F16, tag=f"{dst_tag}{h}",
                               name=dst_tag)
                if pek < NDH:
                    nc.sync.dma_start(
                        xt[:, pek:, :],
                        src[:, pek * 128: NDH * 128], transpose=True,
                    )
                for q in range(pek // GRP):
                    stg = pst.tile([128, GRP * 128], F16, tag="stg",
                                   name="stg")
                    for k in range(GRP):
                        nc.tensor.transpose(
                            stg[:, bass.ts(k, 128)],
                            src[:, bass.ts(q * GRP + k, 128)], identh,
                        )
                    dst = xt[:, q * GRP:(q + 1) * GRP, :]
                    if evict_dve:
                        nc.vector.tensor_copy(dst, stg[:])
                    else:
                        nc.scalar.copy(dst, stg[:])
                return xt

            pending_routing = []
            xhs = x0
            casts = emit_casts(0, x0)

            for i in range(NT):
                xnh, xnl = casts

                # 1. transposes for tile i (PE+ACT evicts / DMA xbar); the
                # xbar dmas are emitted before the next-tile loads so they
                # aren't queued behind 10us of x load on the DMA engines
                pek = hi_pek_tiles.get(i, hi_pek)
                xhT, xlT = [], []
                for h in range(2):
                    hp = 0 if (i, h) in hi_xbar else pek
                    xhT.append(emit_transpose(
                        i, h, xnh[h], "xhT", pek=hp,
                        evict_dve=(i, h) in evict_dve_halves,
                    ))
                    xlT.append(emit_transpose(
                        i, h, xnl[h], "xlT",
                        pek=NDH if (i, h) in lo_pe else 0,
                        evict_dve=(i, h) in evict_dve_halves,
                    ))

                # 2. prefetch next x tile (tile 1 already loaded pre-loop)
                nxt = []
                if i == 0:
                    nxt = x1
                elif i + 1 < NT:
                    for h in range(2):
                        xh_t = xin.tile([128, HALF], F32, tag=f"xn{h}",
                                        name="xn")
                        nc.sync.dma_start(
                            xh_t[:], X[bass.ts(i + 1, 128), bass.ts(h, HALF)]
                        )
                        nxt.append(xh_t)

                # 3. next tile's casts (ACT/DVE run them during our matmuls)
                if nxt:
                    casts = emit_casts(i + 1, nxt)

                # 4. routing for tile i-1 on the DVE
                while pending_routing:
                    pending_routing.pop(0)()

                # 5. matmul sweeps; hi-bank interleaves xh@wh / xh@wl per
                # d-tile and the lo term accumulates in its own PSUM bank
                # (rounding at its small scale), exactly mirroring the
                # accumulation order the reference-matching baseline used
                logits = psl.tile([128, E], F32, tag="logits", name="logits")
                logits_lo = psl.tile([128, E], F32, tag="logits_lo",
                                     name="logits_lo", bufs=1)

                def lo_sweep():
                    for d in range(ND):
                        nc.tensor.matmul(
                            logits_lo[:], xlT[d // NDH][:, d % NDH, :],
                            wht[:, bass.ds(d * E, E)],
                            start=(d == 0), stop=(d == ND - 1),
                        )

                def hi_sweep():
                    for d in range(ND):
                        nc.tensor.matmul(
                            logits[:], xhT[d // NDH][:, d % NDH, :],
                            wht[:, bass.ds(d * E, E)],
                            start=(d == 0), stop=False,
                        )
                        nc.tensor.matmul(
                            logits[:], xhT[d // NDH][:, d % NDH, :],
                            wlt[:, bass.ds(d * E, E)],
                            start=False, stop=(d == ND - 1),
                        )

                # tile 0: lo first (its lo is PE-transposed; the w-lo half
                # is still in flight); steady tiles: hi first so the lo
                # xbar has most of the tile to land
                if i == 0:
                    lo_sweep()
                    hi_sweep()
                else:
                    hi_sweep()
                    lo_sweep()

                # 6. combine banks, sigmoid with descale
                lo_sb = rt.tile([128, E], F32, tag="lo_sb", name="lo_sb",
                                bufs=1)
                nc.scalar.copy(lo_sb[:], logits_lo[:])
                logsum = rt.tile([128, E], F32, tag="logsum", name="logsum",
                                 bufs=1)
                nc.vector.tensor_tensor(logsum[:], logits[:], lo_sb[:],
                                        op=ALU.add)
                scores = rt.tile([128, E], F32, tag="scores", name="scores",
                                 bufs=1)
                nc.scalar.activation(scores[:], logsum[:], AF.Sigmoid,
                                     scale=DESCALE)
                pending_routing = _routing_thunks(nc, rt, scores, biasr,
                                                  OW, OI, i)
                xhs = nxt

            while pending_routing:
                pending_routing.pop(0)()

    nc.compile()
    return nc


def _prep_weights(weight: np.ndarray):
    """Host-side prep of the replicated gate weight: fp16 hi/lo split of
    w*2^12, transposed into the [128, 56, 256] d-major blocked layout the
    matmuls consume."""
    ws = weight.astype(np.float32) * WSCALE
    wh = ws.astype(np.float16)
    wl = (ws - wh.astype(np.float32)).astype(np.float16)

    def blocked(a16):
        # [256, 7168] -> T -> [56, 128, 256] -> [128, 56*256]
        t = np.ascontiguousarray(
            a16.T.reshape(ND, 128, E).transpose(1, 0, 2)
        ).reshape(128, ND * E)
        return t

    return blocked(wh), blocked(wl)


def kernel(x: np.ndarray, weight: np.ndarray, bias: np.ndarray):
    x = np.ascontiguousarray(x, dtype=np.float32)
    weight = np.ascontiguousarray(weight, dtype=np.float32)
    bias = np.ascontiguousarray(bias, dtype=np.float32)

    if "nc" not in _CACHE:
        _CACHE["nc"] = _build(
            lo_pe=frozenset({(0, 0), (0, 1), (1, 0), (1, 1)}),
            hi_pek=28,
        )
    nc = _CACHE["nc"]

    wht, wlt = _prep_weights(weight)
    biasr = np.tile(bias[None, :], (128, 1))
    ident16 = np.eye(128, dtype=np.float16)
    in_maps = [
        {
            "X": x[c * TPC: (c + 1) * TPC],
            "WHT": wht,
            "WLT": wlt,
            "BIASR": biasr,
            "IDENT16": ident16,
        }
        for c in range(N_CORES)
    ]
    res = run_bass_kernel_spmd(nc, in_maps, core_ids=list(range(N_CORES)))
    w = np.concatenate([r["OW"] for r in res.results], axis=0)
    idx = np.concatenate([r["OI"] for r in res.results], axis=0)
    return w, idx
